# revision 13
# baseline (speedup 1.0000x reference)
"""Causal single-head attention (B=4, T=4096, D=1024) on 8 trn2 NeuronCores.

Sharding: 2 cores per batch element, split by key-block PARITY (flash-style):
  core = 2*b + p ; p in {0,1}
  Each core computes, for ALL 4096 queries of batch b, the partial
  (unnormalized) attention output over its 16 key blocks {128*(2u+p)} and the
  partial softmax row-sums. Host merges: O = (O_0 + O_1) / (rs_0 + rs_1).
  exp() without per-row max subtraction (scaled scores are in [-8, 8] for
  randn inputs; exp stays well inside fp32 range).

Per-core on-chip flow (identical program on all 8 cores, data-only differences):
  Phase A: qT = WqT^T-blocks @ xT   -> DRAM scratch qTs [D, T]
  Phase B: kT [d, s] and V [s, d]+ones-col for the core's 16 key blocks
           (SBUF-resident; inputs xTk = parity-gathered x^T cols)
  Phase C: per q-chunk of 256 cols: S^T = kT-blk^T @ qT-chunk (PSUM),
           P^T = exp(S^T/32) (ACT), diagonal/zero mask on last key block,
           O' += P^T-sub^T @ V-blk (PSUM accum, +ones col = row-sums),
           drain O'(+rs) -> DRAM.
All matmuls run as float32r (FP22 single-pass) via bitcast views.
"""

import sys

sys.path.insert(0, "/opt/trn_rl_repo")

import numpy as np
from contextlib import ExitStack

import concourse.tile as tile
from concourse import bacc, mybir
from concourse.bass_utils import run_bass_kernel_spmd

P = 128
D = 1024
T = 4096
B = 4
NDB = D // P  # 8 d-blocks
NCB = D // P  # 8 contraction blocks
NKB = 16  # key blocks per core (parity half of 32)
QC = 256  # query-chunk columns in phase C
NQC = T // QC  # 16
CH = 512  # projection column chunk
F32 = mybir.dt.float32
F32R = mybir.dt.float32r
EXPSCALE = 1.0 / 32.0  # 1/sqrt(D)
EXP = mybir.ActivationFunctionType.Exp

_CACHED_NC = None
_LAST_RES = None


def _build_program():
    nc = bacc.Bacc("TRN2", target_bir_lowering=False, debug=False, num_devices=8)

    xT_d = nc.dram_tensor("xT", [D, T], F32R, kind="ExternalInput").ap()
    xTk_d = nc.dram_tensor("xTk", [D, T // 2], F32R, kind="ExternalInput").ap()
    wq_d = nc.dram_tensor("WqT", [D, D], F32R, kind="ExternalInput").ap()
    wk_d = nc.dram_tensor("WkT", [D, D], F32R, kind="ExternalInput").ap()
    wv_d = nc.dram_tensor("WvT", [D, D], F32R, kind="ExternalInput").ap()
    mask_d = nc.dram_tensor("mask", [P, QC], F32, kind="ExternalInput").ap()
    ones2_d = nc.dram_tensor("ones2", [P, 2], F32R, kind="ExternalInput").ap()
    o_d = nc.dram_tensor("O", [T, D], F32, kind="ExternalOutput").ap()
    rs_d = nc.dram_tensor("rs", [T, 1], F32, kind="ExternalOutput").ap()
    qTs_d = nc.dram_tensor("qTs", [D, T], F32R).ap()  # internal scratch

    xT_r = xT_d.rearrange("(a p) t -> p a t", p=P)  # [128, 8, 4096]
    xTk_r = xTk_d.rearrange("(a p) t -> p a t", p=P)  # [128, 8, 2048]
    wq_r = wq_d.rearrange("(a p) d -> p a d", p=P)  # [128, 8, 1024]
    wk_r = wk_d.rearrange("(a p) d -> p a d", p=P)
    wv_r = wv_d.rearrange("(a p) d -> p a d", p=P)
    qTs_r = qTs_d.rearrange("(a p) t -> p a t", p=P)

    with tile.TileContext(nc) as tc, ExitStack() as ctx:
        kv = ctx.enter_context(tc.tile_pool(name="kv", bufs=1))
        big = ctx.enter_context(tc.tile_pool(name="big", bufs=2))
        wp = ctx.enter_context(tc.tile_pool(name="wp", bufs=2))
        pp = ctx.enter_context(tc.tile_pool(name="pp", bufs=2))
        stg = ctx.enter_context(tc.tile_pool(name="stg", bufs=2))
        cst = ctx.enter_context(tc.tile_pool(name="cst", bufs=1))
        psum = ctx.enter_context(tc.tile_pool(name="psum", bufs=1, space="PSUM"))

        mask_t = cst.tile([P, QC], F32, tag="mask")
        nc.sync.dma_start(mask_t[:], mask_d[:])

        # ---------------- Phase A: qT projection -> DRAM scratch -------------
        for ch in range(T // CH):
            xt = big.tile([P, NCB, CH], F32R, tag="xchunk")
            nc.sync.dma_start(xt[:], xT_r[:, :, ch * CH : (ch + 1) * CH])
            for db in range(NDB):
                wq = wp.tile([P, NCB, P], F32R, tag="wt")
                nc.sync.dma_start(wq[:], wq_r[:, :, db * P : (db + 1) * P])
                ps = psum.tile([P, CH], F32, tag=f"b{db % 4}")
                for cb in range(NCB):
                    nc.tensor.matmul(
                        ps[:],
                        (wq[:, cb, :]),
                        (xt[:, cb, :]),
                        start=(cb == 0),
                        stop=(cb == NCB - 1),
                    )
                st = stg.tile([P, CH], F32R, tag="stage")
                nc.vector.tensor_copy(st[:], ps[:])
                nc.sync.dma_start(qTs_r[:, db, ch * CH : (ch + 1) * CH], st[:])

        # ---------------- Phase B: kT + V (resident) -------------------------
        kt_t = kv.tile([P, NDB, T // 2], F32R, tag="kt")  # [128, 8, 2048]
        v_t = kv.tile([P, NKB, D + 2], F32R, tag="vt")  # [128, 16, 1026]
        for g in range(4):  # groups of 4 key blocks (512 cols of xTk)
            xk = big.tile([P, NCB, CH], F32R, tag="xchunk")
            nc.sync.dma_start(xk[:], xTk_r[:, :, g * CH : (g + 1) * CH])
            for db in range(NDB):
                wk = wp.tile([P, NCB, P], F32R, tag="wt")
                nc.sync.dma_start(wk[:], wk_r[:, :, db * P : (db + 1) * P])
                ps = psum.tile([P, CH], F32, tag=f"b{4 + db % 2}")
                for cb in range(NCB):
                    nc.tensor.matmul(
                        ps[:],
                        (wk[:, cb, :]),
                        (xk[:, cb, :]),
                        start=(cb == 0),
                        stop=(cb == NCB - 1),
                    )
                nc.vector.tensor_copy(kt_t[:, db, g * CH : (g + 1) * CH], ps[:])
            for vc in range(2):  # 512-col chunks of V's d dim
                wv = wp.tile([P, NCB, 512], F32R, tag="wv")
                nc.sync.dma_start(wv[:], wv_r[:, :, vc * 512 : (vc + 1) * 512])
                for i in range(4):
                    kb = 4 * g + i
                    ps = psum.tile([P, 512], F32, tag=f"b{6 + i % 2}")
                    for cb in range(NCB):
                        nc.tensor.matmul(
                            ps[:],
                            (xk[:, cb, i * P : (i + 1) * P]),
                            (wv[:, cb, :]),
                            start=(cb == 0),
                            stop=(cb == NCB - 1),
                        )
                    nc.vector.tensor_copy(v_t[:, kb, vc * 512 : (vc + 1) * 512], ps[:])
        for kb in range(NKB):
            nc.sync.dma_start(v_t[:, kb, D : D + 2], ones2_d[:])

        # ---------------- Phase C: attention ---------------------------------
        for j in reversed(range(NQC)):
            qt = big.tile([P, NDB, QC], F32R, tag="xchunk")
            nc.sync.dma_start(qt[:], qTs_r[:, :, j * QC : (j + 1) * QC])
            acc = {}
            for sub in range(2):
                for c in range(3):
                    shape = [P, 2] if c == 2 else [P, 512]
                    acc[sub, c] = psum.tile(
                        shape, F32, tag=f"b{sub * 3 + c}", name=f"acc{j}_{sub}_{c}"
                    )
            for u in range(j + 1):
                st = psum.tile([P, QC], F32, tag=f"b{6 + u % 2}")
                for db in range(NDB):
                    nc.tensor.matmul(
                        st[:],
                        (kt_t[:, db, u * P : (u + 1) * P]),
                        (qt[:, db, :]),
                        start=(db == 0),
                        stop=(db == NDB - 1),
                    )
                if u == j:
                    nc.vector.tensor_add(st[:], st[:], mask_t[:])
                pt = pp.tile([P, QC], F32R, tag="pt")
                nc.scalar.activation(pt[:], st[:], EXP, scale=EXPSCALE)
                first, last = (u == 0), (u == j)
                for sub in range(2):
                    lhs = (pt[:, sub * P : (sub + 1) * P])
                    nc.tensor.matmul(
                        acc[sub, 0][:], lhs, (v_t[:, u, 0:512]),
                        start=first, stop=last, skip_group_check=True,
                    )
                    nc.tensor.matmul(
                        acc[sub, 1][:], lhs, (v_t[:, u, 512:1024]),
                        start=first, stop=last, skip_group_check=True,
                    )
                    nc.tensor.matmul(
                        acc[sub, 2][:], lhs, (v_t[:, u, D : D + 2]),
                        start=first, stop=last, skip_group_check=True,
                    )
            for sub in range(2):
                row = j * QC + sub * P
                ot0 = stg.tile([P, 512], F32, tag="stage")
                nc.vector.tensor_copy(ot0[:], acc[sub, 0][:])
                ot1 = stg.tile([P, 512], F32, tag="stage")
                nc.vector.tensor_copy(ot1[:], acc[sub, 1][:])
                rt = stg.tile([P, 1], F32, tag="rt")
                nc.vector.tensor_copy(rt[:], acc[sub, 2][:, 0:1])
                nc.sync.dma_start(o_d[row : row + P, 0:512], ot0[:])
                nc.sync.dma_start(o_d[row : row + P, 512:1024], ot1[:])
                nc.sync.dma_start(rs_d[row : row + P, :], rt[:])

    nc.finalize()
    return nc


def _get_program():
    global _CACHED_NC
    if _CACHED_NC is None:
        _CACHED_NC = _build_program()
    return _CACHED_NC


def _masks():
    neg = np.float32(-1e30)
    tri = np.where(np.triu(np.ones((P, P), dtype=bool)), np.float32(0), neg)
    keep = np.zeros((P, P), dtype=np.float32)
    drop = np.full((P, P), neg, dtype=np.float32)
    return (
        np.ascontiguousarray(np.concatenate([tri, keep], axis=1)),  # even core
        np.ascontiguousarray(np.concatenate([drop, tri], axis=1)),  # odd core
    )


def kernel(x, Wq, Wk, Wv):
    out, _ = _run(x, Wq, Wk, Wv, trace=False)
    return out


def _run(x, Wq, Wk, Wv, trace=False, keep_res=False):
    x = np.ascontiguousarray(np.asarray(x, dtype=np.float32))
    WqT = np.ascontiguousarray(np.asarray(Wq, dtype=np.float32).T)
    WkT = np.ascontiguousarray(np.asarray(Wk, dtype=np.float32).T)
    WvT = np.ascontiguousarray(np.asarray(Wv, dtype=np.float32).T)
    m_even, m_odd = _masks()
    ones2 = np.ascontiguousarray(
        np.repeat(np.array([[1.0, 0.0]], dtype=np.float32), P, axis=0)
    )

    nc = _get_program()
    in_maps = []
    for core in range(8):
        b, p = core // 2, core % 2
        xT = np.ascontiguousarray(x[b].T)  # [D, T]
        xTk = np.ascontiguousarray(
            xT.reshape(D, T // P, P)[:, p::2, :].reshape(D, T // 2)
        )
        in_maps.append(
            {
                "xT": xT,
                "xTk": xTk,
                "WqT": WqT,
                "WkT": WkT,
                "WvT": WvT,
                "mask": m_even if p == 0 else m_odd,
                "ones2": ones2,
            }
        )

    res = run_bass_kernel_spmd(nc, in_maps, core_ids=list(range(8)), trace=trace)
    if keep_res:
        global _LAST_RES
        _LAST_RES = res
    out = np.empty((B, T, D), dtype=np.float32)
    for b in range(B):
        O0, rs0 = res.results[2 * b]["O"], res.results[2 * b]["rs"]
        O1, rs1 = res.results[2 * b + 1]["O"], res.results[2 * b + 1]["rs"]
        out[b] = (O0 + O1) / (rs0 + rs1)
    return out, res.exec_time_ns
